# revision 3
# baseline (speedup 1.0000x reference)
"""Single-head attention (B=4, S=2048, D=1024) on 8 TRN2 NeuronCores.

Sharding: each core handles one (batch, query-half) pair -> 8 shards of
1024 query rows. K/V projections for the full batch sequence are computed
redundantly on both cores of a batch pair (v1: no collectives).

Layout trick: everything flows transposed so no on-chip transposes needed.
  - host feeds x^T tiles [d_in, rows]
  - Q/K projections produce [d_out, rows] (= proj^T) via lhsT=weight
  - scores^T [k, q] = (K^T as lhsT).T-free matmul with rhs=Q^T
  - softmax denominator comes free from an extra ones-column in the AV
    matmul; normalization + V-bias fused into the output eviction
    (out = attn@(Vraw+bv) = (exp@Vraw)/sums + bv since rows of attn sum to 1).
  - exp() needs no max-subtraction: scores are bounded (~|2.3| max) by
    construction of the inputs.
Compute dtype bf16 (PE full rate), fp32 PSUM accumulation, fp32 output.
"""

import sys

import numpy as np

try:
    import concourse  # noqa: F401
except ImportError:  # pragma: no cover
    sys.path.insert(0, "/opt/trn_rl_repo")

import ml_dtypes

import concourse.bass as bass  # noqa: F401
import concourse.mybir as mybir
import concourse.tile as tile
from concourse import bacc
from concourse.bass import ds, ts
from concourse.bass_utils import run_bass_kernel_spmd

P = 128          # partitions
D = 1024         # embed dim
S = 2048         # sequence length
B = 4            # batch
QH = S // 2      # query rows per core
NCORES = 8
DJ = D // P      # 8  d-tiles
KJ = S // P      # 16 k/s-tiles
QJ = QH // P     # 8  q-tiles
NCH = 512        # moving-operand chunk (one PSUM bank of fp32)
SCALE = 1.0 / 32.0  # 1/sqrt(D)

DT = mybir.dt.bfloat16
F32 = mybir.dt.float32
NPDT = ml_dtypes.bfloat16

AF = mybir.ActivationFunctionType
OP = mybir.AluOpType


def build():
    nc = bacc.Bacc("TRN2", target_bir_lowering=False, debug=False,
                   num_devices=NCORES)

    qT_d = nc.dram_tensor("qT", [D, QH], DT, kind="ExternalInput").ap()
    kT_d = nc.dram_tensor("kT", [D, S], DT, kind="ExternalInput").ap()
    vT_d = nc.dram_tensor("vT", [D, S], DT, kind="ExternalInput").ap()
    wq_d = nc.dram_tensor("wq", [D, D], DT, kind="ExternalInput").ap()
    wk_d = nc.dram_tensor("wk", [D, D], DT, kind="ExternalInput").ap()
    wv_d = nc.dram_tensor("wv", [D, D], DT, kind="ExternalInput").ap()
    bq_d = nc.dram_tensor("bqc", [P, DJ], F32, kind="ExternalInput").ap()
    bk_d = nc.dram_tensor("bkc", [P, DJ], F32, kind="ExternalInput").ap()
    bv_d = nc.dram_tensor("bvb", [P, D], F32, kind="ExternalInput").ap()
    out_d = nc.dram_tensor("out", [QH, D], F32, kind="ExternalOutput").ap()

    def part3(ap):  # [(n p), d] -> [p, n, d]
        return ap.rearrange("(n p) d -> p n d", p=P)

    with tile.TileContext(nc) as tc:
        with (
            tc.tile_pool(name="persist", bufs=1) as pp,
            tc.tile_pool(name="xin", bufs=1) as xp,
            tc.tile_pool(name="win", bufs=1) as wp,
            tc.tile_pool(name="ev", bufs=3) as ep,
            tc.tile_pool(name="psum", bufs=2, space="PSUM") as psp,
        ):
            # constants
            bq_t = pp.tile([P, DJ], F32, tag="bq")
            nc.sync.dma_start(bq_t[:], bq_d[:])
            bk_t = pp.tile([P, DJ], F32, tag="bk")
            nc.sync.dma_start(bk_t[:], bk_d[:])
            bv_t = pp.tile([P, D], F32, tag="bv")
            nc.sync.dma_start(bv_t[:], bv_d[:])
            ones_t = pp.tile([P, 1], DT, tag="ones")
            nc.vector.memset(ones_t[:], 1.0)

            # persistent intermediates
            qT_proj = pp.tile([P, DJ, QH], DT, tag="qproj")   # (Q+bq)^T / 32
            kT_proj = pp.tile([P, DJ, S], DT, tag="kproj")    # (K+bk)^T
            expT = pp.tile([P, KJ, QH], DT, tag="expT")       # exp(scores)^T
            v_nat = pp.tile([P, KJ, D], DT, tag="vnat")       # value@wv (no bias)

            # ---- P1: Q projection -> qT_proj [d_out, q]
            wq_t = wp.tile([P, DJ, D], DT, tag="w")
            nc.sync.dma_start(wq_t[:], part3(wq_d))
            qT_in = xp.tile([P, DJ, QH], DT, tag="x")
            nc.sync.dma_start(qT_in[:], part3(qT_d))
            for do in range(DJ):
                ps0 = psp.tile([P, NCH], F32, tag="psA")
                ps1 = psp.tile([P, NCH], F32, tag="psB")
                for di in range(DJ):
                    w_ap = wq_t[:, di, ts(do, P)]
                    nc.tensor.matmul(ps0[:], w_ap, qT_in[:, di, ds(0, NCH)],
                                     start=(di == 0), stop=(di == DJ - 1))
                    nc.tensor.matmul(ps1[:], w_ap, qT_in[:, di, ds(NCH, NCH)],
                                     start=(di == 0), stop=(di == DJ - 1))
                nc.vector.tensor_scalar(qT_proj[:, do, ds(0, NCH)], ps0[:],
                                        bq_t[:, ds(do, 1)], SCALE, OP.add, OP.mult)
                nc.vector.tensor_scalar(qT_proj[:, do, ds(NCH, NCH)], ps1[:],
                                        bq_t[:, ds(do, 1)], SCALE, OP.add, OP.mult)

            # ---- P2: K projection (full seq) -> kT_proj [d_out, s]
            wk_t = wp.tile([P, DJ, D], DT, tag="w")
            nc.sync.dma_start(wk_t[:], part3(wk_d))
            kT_in = xp.tile([P, DJ, S], DT, tag="x")
            nc.sync.dma_start(kT_in[:], part3(kT_d))
            for do in range(DJ):
                for half in range(2):
                    ps0 = psp.tile([P, NCH], F32, tag="psA")
                    ps1 = psp.tile([P, NCH], F32, tag="psB")
                    for di in range(DJ):
                        w_ap = wk_t[:, di, ts(do, P)]
                        nc.tensor.matmul(ps0[:], w_ap,
                                         kT_in[:, di, ds(half * 1024, NCH)],
                                         start=(di == 0), stop=(di == DJ - 1))
                        nc.tensor.matmul(ps1[:], w_ap,
                                         kT_in[:, di, ds(half * 1024 + NCH, NCH)],
                                         start=(di == 0), stop=(di == DJ - 1))
                    nc.vector.tensor_scalar_add(
                        kT_proj[:, do, ds(half * 1024, NCH)], ps0[:],
                        bk_t[:, ds(do, 1)])
                    nc.vector.tensor_scalar_add(
                        kT_proj[:, do, ds(half * 1024 + NCH, NCH)], ps1[:],
                        bk_t[:, ds(do, 1)])

            # ---- P3: scores^T + exp -> expT [k, q]
            for kt in range(KJ):
                ps0 = psp.tile([P, NCH], F32, tag="psA")
                ps1 = psp.tile([P, NCH], F32, tag="psB")
                for di in range(DJ):
                    k_ap = kT_proj[:, di, ts(kt, P)]
                    nc.tensor.matmul(ps0[:], k_ap, qT_proj[:, di, ds(0, NCH)],
                                     start=(di == 0), stop=(di == DJ - 1))
                    nc.tensor.matmul(ps1[:], k_ap, qT_proj[:, di, ds(NCH, NCH)],
                                     start=(di == 0), stop=(di == DJ - 1))
                nc.scalar.activation(expT[:, kt, ds(0, NCH)], ps0[:], AF.Exp)
                nc.scalar.activation(expT[:, kt, ds(NCH, NCH)], ps1[:], AF.Exp)

            # ---- P4: V projection (full seq, natural layout, no bias)
            wv_t = wp.tile([P, DJ, D], DT, tag="w")
            nc.sync.dma_start(wv_t[:], part3(wv_d))
            vT_in = xp.tile([P, DJ, S], DT, tag="x")
            nc.sync.dma_start(vT_in[:], part3(vT_d))
            for st in range(KJ):
                ps0 = psp.tile([P, NCH], F32, tag="psA")
                ps1 = psp.tile([P, NCH], F32, tag="psB")
                for di in range(DJ):
                    v_ap = vT_in[:, di, ts(st, P)]
                    nc.tensor.matmul(ps0[:], v_ap, wv_t[:, di, ds(0, NCH)],
                                     start=(di == 0), stop=(di == DJ - 1))
                    nc.tensor.matmul(ps1[:], v_ap, wv_t[:, di, ds(NCH, NCH)],
                                     start=(di == 0), stop=(di == DJ - 1))
                nc.vector.tensor_copy(v_nat[:, st, ds(0, NCH)], ps0[:])
                nc.vector.tensor_copy(v_nat[:, st, ds(NCH, NCH)], ps1[:])

            # ---- P5: AV + fused normalize/bias -> out
            for qt in range(QJ):
                po0 = psp.tile([P, NCH], F32, tag="psA")
                po1 = psp.tile([P, NCH], F32, tag="psB")
                psm = psp.tile([P, 1], F32, tag="psS")
                for kt in range(KJ):
                    e_ap = expT[:, kt, ts(qt, P)]
                    nc.tensor.matmul(po0[:], e_ap, v_nat[:, kt, ds(0, NCH)],
                                     start=(kt == 0), stop=(kt == KJ - 1))
                    nc.tensor.matmul(po1[:], e_ap, v_nat[:, kt, ds(NCH, NCH)],
                                     start=(kt == 0), stop=(kt == KJ - 1))
                    nc.tensor.matmul(psm[:], e_ap, ones_t[:],
                                     start=(kt == 0), stop=(kt == KJ - 1))
                recip = ep.tile([P, 1], F32, tag="recip")
                nc.vector.reciprocal(recip[:], psm[:])
                ot = ep.tile([P, D], F32, tag="out")
                nc.vector.scalar_tensor_tensor(
                    ot[:, ds(0, NCH)], po0[:], recip[:], bv_t[:, ds(0, NCH)],
                    OP.mult, OP.add)
                nc.vector.scalar_tensor_tensor(
                    ot[:, ds(NCH, NCH)], po1[:], recip[:], bv_t[:, ds(NCH, NCH)],
                    OP.mult, OP.add)
                nc.sync.dma_start(out_d[ts(qt, P), :], ot[:])

    nc.compile()
    return nc


_NC = None


def _get_nc():
    global _NC
    if _NC is None:
        _NC = build()
    return _NC


def _install_profile_hook():
    """The agent image's `antenv` lacks `axon_hooks`, so the boot-time NTFF
    profile hook install degrades silently. Recreate the registry module and
    install the ctypes-based hook so trace=True yields exec_time_ns."""
    import types
    try:
        from antenv.axon_hooks import get_axon_ntff_profile_hook  # noqa: F401
        return  # already present
    except ImportError:
        pass
    import antenv
    mod = types.ModuleType("antenv.axon_hooks")
    _hook = [None]
    mod.set_axon_ntff_profile_hook = lambda h: _hook.__setitem__(0, h)
    mod.get_axon_ntff_profile_hook = lambda: _hook[0]
    sys.modules["antenv.axon_hooks"] = mod
    antenv.axon_hooks = mod
    sys.path.insert(0, "/root/.axon_site")
    from trn_agent_boot.trn_boot import _ntff_profile_via_ctypes
    mod.set_axon_ntff_profile_hook(
        _ntff_profile_via_ctypes("/opt/axon/libaxon_pjrt.so"))


def _prep_in_maps(inputs):
    f32 = np.float32
    q = np.asarray(inputs["query"], f32)
    k = np.asarray(inputs["key"], f32)
    v = np.asarray(inputs["value"], f32)
    wq = np.ascontiguousarray(np.asarray(inputs["wq"], f32).astype(NPDT))
    wk = np.ascontiguousarray(np.asarray(inputs["wk"], f32).astype(NPDT))
    wv = np.ascontiguousarray(np.asarray(inputs["wv"], f32).astype(NPDT))
    bq = np.ascontiguousarray(np.asarray(inputs["bq"], f32).reshape(DJ, P).T)
    bk = np.ascontiguousarray(np.asarray(inputs["bk"], f32).reshape(DJ, P).T)
    bv = np.ascontiguousarray(
        np.broadcast_to(np.asarray(inputs["bv"], f32), (P, D)))

    in_maps = []
    for c in range(NCORES):
        b, h = divmod(c, 2)
        qT = np.ascontiguousarray(
            q[b, h * QH:(h + 1) * QH, :].astype(NPDT).T)
        kT = np.ascontiguousarray(k[b].astype(NPDT).T)
        vT = np.ascontiguousarray(v[b].astype(NPDT).T)
        in_maps.append({
            "qT": qT, "kT": kT, "vT": vT,
            "wq": wq, "wk": wk, "wv": wv,
            "bqc": bq, "bkc": bk, "bvb": bv,
        })
    return in_maps


def run(inputs, trace=False):
    """Returns (full_output [B,S,D] fp32, exec_time_ns or None)."""
    nc = _get_nc()
    in_maps = _prep_in_maps(inputs)
    if trace:
        _install_profile_hook()
    res = run_bass_kernel_spmd(nc, in_maps, list(range(NCORES)), trace=trace)
    out = np.empty((B, S, D), np.float32)
    for c in range(NCORES):
        b, h = divmod(c, 2)
        out[b, h * QH:(h + 1) * QH, :] = res.results[c]["out"]
    return out, res.exec_time_ns


def kernel(**inputs):
    return run(inputs, trace=False)[0]


# revision 7
# speedup vs baseline: 1.0946x; 1.0946x over previous
"""Single-head attention (B=4, S=2048, D=1024) on 8 TRN2 NeuronCores.

Sharding: each core handles one (batch, query-half) pair -> 8 shards of
1024 query rows. K/V projections for the full batch sequence are computed
redundantly on both cores of a batch pair (v1: no collectives).

Layout trick: everything flows transposed so no on-chip transposes needed.
  - host feeds x^T tiles [d_in, rows]
  - Q/K projections produce [d_out, rows] (= proj^T) via lhsT=weight
  - scores^T [k, q] = (K^T as lhsT).T-free matmul with rhs=Q^T
  - softmax denominator comes free from an extra ones-column in the AV
    matmul; normalization + V-bias fused into the output eviction
    (out = attn@(Vraw+bv) = (exp@Vraw)/sums + bv since rows of attn sum to 1).
  - exp() needs no max-subtraction: scores are bounded (~|2.3| max) by
    construction of the inputs.
Compute dtype bf16 (PE full rate), fp32 PSUM accumulation, fp32 output.
"""

import sys

import numpy as np

try:
    import concourse  # noqa: F401
except ImportError:  # pragma: no cover
    sys.path.insert(0, "/opt/trn_rl_repo")

import ml_dtypes

import concourse.bass as bass  # noqa: F401
import concourse.mybir as mybir
import concourse.tile as tile
from concourse import bacc
from concourse.bass import ds, ts
from concourse.bass_utils import run_bass_kernel_spmd

P = 128          # partitions
D = 1024         # embed dim
S = 2048         # sequence length
B = 4            # batch
QH = S // 2      # query rows per core
NCORES = 8
DJ = D // P      # 8  d-tiles
KJ = S // P      # 16 k/s-tiles
QJ = QH // P     # 8  q-tiles
NCH = 512        # moving-operand chunk (one PSUM bank of fp32)
SCALE = 1.0 / 32.0  # 1/sqrt(D)

DT = mybir.dt.bfloat16
F32 = mybir.dt.float32
NPDT = ml_dtypes.bfloat16

AF = mybir.ActivationFunctionType
OP = mybir.AluOpType


def build():
    nc = bacc.Bacc("TRN2", target_bir_lowering=False, debug=False,
                   num_devices=NCORES)

    qT_d = nc.dram_tensor("qT", [D, QH], DT, kind="ExternalInput").ap()
    kT_d = nc.dram_tensor("kT", [D, S], DT, kind="ExternalInput").ap()
    vT_d = nc.dram_tensor("vT", [D, S], DT, kind="ExternalInput").ap()
    wq_d = nc.dram_tensor("wq", [D, D], DT, kind="ExternalInput").ap()
    wk_d = nc.dram_tensor("wk", [D, D], DT, kind="ExternalInput").ap()
    wv_d = nc.dram_tensor("wv", [D, D], DT, kind="ExternalInput").ap()
    bq_d = nc.dram_tensor("bqc", [P, DJ], F32, kind="ExternalInput").ap()
    bk_d = nc.dram_tensor("bkc", [P, DJ], F32, kind="ExternalInput").ap()
    bv_d = nc.dram_tensor("bvb", [P, D], F32, kind="ExternalInput").ap()
    out_d = nc.dram_tensor("out", [QH, D], F32, kind="ExternalOutput").ap()

    def part3(ap):  # [(n p), d] -> [p, n, d]
        return ap.rearrange("(n p) d -> p n d", p=P)

    with tile.TileContext(nc) as tc:
        with (
            tc.tile_pool(name="persist", bufs=1) as pp,
            tc.tile_pool(name="xin", bufs=2) as xp,
            tc.tile_pool(name="win", bufs=2) as wp,
            tc.tile_pool(name="ev", bufs=2) as ep,
            tc.tile_pool(name="psum", bufs=2, space="PSUM") as psp,
        ):
            # constants
            bq_t = pp.tile([P, DJ], F32, tag="bq")
            nc.sync.dma_start(bq_t[:], bq_d[:])
            bk_t = pp.tile([P, DJ], F32, tag="bk")
            nc.sync.dma_start(bk_t[:], bk_d[:])
            bv_t = pp.tile([P, D], F32, tag="bv")
            nc.sync.dma_start(bv_t[:], bv_d[:])
            ones_t = pp.tile([P, 1], DT, tag="ones")
            nc.vector.memset(ones_t[:], 1.0)

            # persistent intermediates
            qT_proj = pp.tile([P, DJ, QH], DT, tag="qproj")   # (Q+bq)^T / 32
            kT_proj = pp.tile([P, DJ, S], DT, tag="kproj")    # (K+bk)^T
            expT = pp.tile([P, KJ, QH], DT, tag="expT")       # exp(scores)^T
            v_nat = pp.tile([P, KJ, D], DT, tag="vnat")       # value@wv (no bias)

            def load_split(tl, src):
                # one dma_start per d-tile so the loads spread across DMA
                # queues and the first matmul can start after ~256KB
                for di in range(DJ):
                    nc.sync.dma_start(tl[:, di, :], src[:, di, :])

            # ---- P1: Q projection -> qT_proj [d_out, q]
            wq_t = wp.tile([P, DJ, D], DT, tag="w")
            load_split(wq_t, part3(wq_d))
            qT_in = xp.tile([P, DJ, QH], DT, tag="x")
            load_split(qT_in, part3(qT_d))
            for do in range(DJ):
                ps0 = psp.tile([P, NCH], F32, tag="psA")
                ps1 = psp.tile([P, NCH], F32, tag="psB")
                for di in range(DJ):
                    w_ap = wq_t[:, di, ts(do, P)]
                    nc.tensor.matmul(ps0[:], w_ap, qT_in[:, di, ds(0, NCH)],
                                     start=(di == 0), stop=(di == DJ - 1))
                    nc.tensor.matmul(ps1[:], w_ap, qT_in[:, di, ds(NCH, NCH)],
                                     start=(di == 0), stop=(di == DJ - 1))
                nc.vector.tensor_scalar(qT_proj[:, do, ds(0, NCH)], ps0[:],
                                        bq_t[:, ds(do, 1)], SCALE, OP.add, OP.mult)
                nc.vector.tensor_scalar(qT_proj[:, do, ds(NCH, NCH)], ps1[:],
                                        bq_t[:, ds(do, 1)], SCALE, OP.add, OP.mult)

            # ---- P2: K projection (full seq) -> kT_proj [d_out, s]
            # processed in sequence halves so the "x" input slots stay small
            # (QH-sized) and next-half loads overlap current-half compute
            wk_t = wp.tile([P, DJ, D], DT, tag="w")
            load_split(wk_t, part3(wk_d))
            kT_r = part3(kT_d)
            for half in range(2):
                kT_in = xp.tile([P, DJ, QH], DT, tag="x")
                for di in range(DJ):
                    nc.sync.dma_start(kT_in[:, di, :],
                                      kT_r[:, di, ds(half * QH, QH)])
                for do in range(DJ):
                    ps0 = psp.tile([P, NCH], F32, tag="psA")
                    ps1 = psp.tile([P, NCH], F32, tag="psB")
                    for di in range(DJ):
                        w_ap = wk_t[:, di, ts(do, P)]
                        nc.tensor.matmul(ps0[:], w_ap, kT_in[:, di, ds(0, NCH)],
                                         start=(di == 0), stop=(di == DJ - 1))
                        nc.tensor.matmul(ps1[:], w_ap, kT_in[:, di, ds(NCH, NCH)],
                                         start=(di == 0), stop=(di == DJ - 1))
                    nc.vector.tensor_scalar_add(
                        kT_proj[:, do, ds(half * 1024, NCH)], ps0[:],
                        bk_t[:, ds(do, 1)])
                    nc.vector.tensor_scalar_add(
                        kT_proj[:, do, ds(half * 1024 + NCH, NCH)], ps1[:],
                        bk_t[:, ds(do, 1)])

            # ---- P3: scores^T + exp -> expT [k, q]
            for kt in range(KJ):
                ps0 = psp.tile([P, NCH], F32, tag="psA")
                ps1 = psp.tile([P, NCH], F32, tag="psB")
                for di in range(DJ):
                    k_ap = kT_proj[:, di, ts(kt, P)]
                    nc.tensor.matmul(ps0[:], k_ap, qT_proj[:, di, ds(0, NCH)],
                                     start=(di == 0), stop=(di == DJ - 1))
                    nc.tensor.matmul(ps1[:], k_ap, qT_proj[:, di, ds(NCH, NCH)],
                                     start=(di == 0), stop=(di == DJ - 1))
                nc.scalar.activation(expT[:, kt, ds(0, NCH)], ps0[:], AF.Exp)
                nc.scalar.activation(expT[:, kt, ds(NCH, NCH)], ps1[:], AF.Exp)

            # ---- P4: V projection (full seq, natural layout, no bias)
            wv_t = wp.tile([P, DJ, D], DT, tag="w")
            load_split(wv_t, part3(wv_d))
            vT_r = part3(vT_d)
            for half in range(2):
                vT_in = xp.tile([P, DJ, QH], DT, tag="x")
                for di in range(DJ):
                    nc.sync.dma_start(vT_in[:, di, :],
                                      vT_r[:, di, ds(half * QH, QH)])
                for st in range(KJ // 2):
                    sg = half * (KJ // 2) + st
                    ps0 = psp.tile([P, NCH], F32, tag="psA")
                    ps1 = psp.tile([P, NCH], F32, tag="psB")
                    for di in range(DJ):
                        v_ap = vT_in[:, di, ts(st, P)]
                        nc.tensor.matmul(ps0[:], v_ap, wv_t[:, di, ds(0, NCH)],
                                         start=(di == 0), stop=(di == DJ - 1))
                        nc.tensor.matmul(ps1[:], v_ap, wv_t[:, di, ds(NCH, NCH)],
                                         start=(di == 0), stop=(di == DJ - 1))
                    nc.vector.tensor_copy(v_nat[:, sg, ds(0, NCH)], ps0[:])
                    nc.vector.tensor_copy(v_nat[:, sg, ds(NCH, NCH)], ps1[:])

            # ---- P5: AV + fused normalize/bias -> out
            for qt in range(QJ):
                po0 = psp.tile([P, NCH], F32, tag="psA")
                po1 = psp.tile([P, NCH], F32, tag="psB")
                psm = psp.tile([P, 1], F32, tag="psS")
                for kt in range(KJ):
                    e_ap = expT[:, kt, ts(qt, P)]
                    nc.tensor.matmul(po0[:], e_ap, v_nat[:, kt, ds(0, NCH)],
                                     start=(kt == 0), stop=(kt == KJ - 1))
                    nc.tensor.matmul(po1[:], e_ap, v_nat[:, kt, ds(NCH, NCH)],
                                     start=(kt == 0), stop=(kt == KJ - 1))
                    nc.tensor.matmul(psm[:], e_ap, ones_t[:],
                                     start=(kt == 0), stop=(kt == KJ - 1))
                recip = ep.tile([P, 1], F32, tag="recip")
                nc.vector.reciprocal(recip[:], psm[:])
                ot = ep.tile([P, D], F32, tag="out")
                nc.vector.scalar_tensor_tensor(
                    ot[:, ds(0, NCH)], po0[:], recip[:], bv_t[:, ds(0, NCH)],
                    OP.mult, OP.add)
                nc.vector.scalar_tensor_tensor(
                    ot[:, ds(NCH, NCH)], po1[:], recip[:], bv_t[:, ds(NCH, NCH)],
                    OP.mult, OP.add)
                nc.sync.dma_start(out_d[ts(qt, P), :], ot[:])

    nc.compile()
    return nc


_NC = None


def _get_nc():
    global _NC
    if _NC is None:
        _NC = build()
    return _NC


def _install_profile_hook():
    """The agent image's `antenv` lacks `axon_hooks`, so the boot-time NTFF
    profile hook install degrades silently. Recreate the registry module and
    install the ctypes-based hook so trace=True yields exec_time_ns."""
    import types
    try:
        from antenv.axon_hooks import get_axon_ntff_profile_hook  # noqa: F401
        return  # already present
    except ImportError:
        pass
    import antenv
    mod = types.ModuleType("antenv.axon_hooks")
    _hook = [None]
    mod.set_axon_ntff_profile_hook = lambda h: _hook.__setitem__(0, h)
    mod.get_axon_ntff_profile_hook = lambda: _hook[0]
    sys.modules["antenv.axon_hooks"] = mod
    antenv.axon_hooks = mod
    sys.path.insert(0, "/root/.axon_site")
    from trn_agent_boot.trn_boot import _ntff_profile_via_ctypes
    mod.set_axon_ntff_profile_hook(
        _ntff_profile_via_ctypes("/opt/axon/libaxon_pjrt.so"))


def _prep_in_maps(inputs):
    f32 = np.float32
    q = np.asarray(inputs["query"], f32)
    k = np.asarray(inputs["key"], f32)
    v = np.asarray(inputs["value"], f32)
    wq = np.ascontiguousarray(np.asarray(inputs["wq"], f32).astype(NPDT))
    wk = np.ascontiguousarray(np.asarray(inputs["wk"], f32).astype(NPDT))
    wv = np.ascontiguousarray(np.asarray(inputs["wv"], f32).astype(NPDT))
    bq = np.ascontiguousarray(np.asarray(inputs["bq"], f32).reshape(DJ, P).T)
    bk = np.ascontiguousarray(np.asarray(inputs["bk"], f32).reshape(DJ, P).T)
    bv = np.ascontiguousarray(
        np.broadcast_to(np.asarray(inputs["bv"], f32), (P, D)))

    in_maps = []
    for c in range(NCORES):
        b, h = divmod(c, 2)
        qT = np.ascontiguousarray(
            q[b, h * QH:(h + 1) * QH, :].astype(NPDT).T)
        kT = np.ascontiguousarray(k[b].astype(NPDT).T)
        vT = np.ascontiguousarray(v[b].astype(NPDT).T)
        in_maps.append({
            "qT": qT, "kT": kT, "vT": vT,
            "wq": wq, "wk": wk, "wv": wv,
            "bqc": bq, "bkc": bk, "bvb": bv,
        })
    return in_maps


def run(inputs, trace=False):
    """Returns (full_output [B,S,D] fp32, exec_time_ns or None)."""
    nc = _get_nc()
    in_maps = _prep_in_maps(inputs)
    if trace:
        _install_profile_hook()
    res = run_bass_kernel_spmd(nc, in_maps, list(range(NCORES)), trace=trace)
    out = np.empty((B, S, D), np.float32)
    for c in range(NCORES):
        b, h = divmod(c, 2)
        out[b, h * QH:(h + 1) * QH, :] = res.results[c]["out"]
    return out, res.exec_time_ns


def kernel(**inputs):
    return run(inputs, trace=False)[0]


# revision 10
# speedup vs baseline: 1.0990x; 1.0041x over previous
"""Single-head attention (B=4, S=2048, D=1024) on 8 TRN2 NeuronCores.

Sharding: each core handles one (batch, query-half) pair -> 8 shards of
1024 query rows. K/V projections for the full batch sequence are computed
redundantly on both cores of a batch pair (v1: no collectives).

Layout trick: everything flows transposed so no on-chip transposes needed.
  - host feeds x^T tiles [d_in, rows]
  - Q/K projections produce [d_out, rows] (= proj^T) via lhsT=weight
  - scores^T [k, q] = (K^T as lhsT).T-free matmul with rhs=Q^T
  - softmax denominator comes free from an extra ones-column in the AV
    matmul; normalization + V-bias fused into the output eviction
    (out = attn@(Vraw+bv) = (exp@Vraw)/sums + bv since rows of attn sum to 1).
  - exp() needs no max-subtraction: scores are bounded (~|2.3| max) by
    construction of the inputs.
Compute dtype bf16 (PE full rate), fp32 PSUM accumulation, fp32 output.
"""

import sys

import numpy as np

try:
    import concourse  # noqa: F401
except ImportError:  # pragma: no cover
    sys.path.insert(0, "/opt/trn_rl_repo")

import ml_dtypes

import concourse.bass as bass  # noqa: F401
import concourse.mybir as mybir
import concourse.tile as tile
from concourse import bacc
from concourse.bass import ds, ts
from concourse.bass_utils import run_bass_kernel_spmd

P = 128          # partitions
D = 1024         # embed dim
S = 2048         # sequence length
B = 4            # batch
QH = S // 2      # query rows per core
NCORES = 8
DJ = D // P      # 8  d-tiles
KJ = S // P      # 16 k/s-tiles
QJ = QH // P     # 8  q-tiles
NCH = 512        # moving-operand chunk (one PSUM bank of fp32)
SCALE = 1.0 / 32.0  # 1/sqrt(D)

DT = mybir.dt.bfloat16
F32 = mybir.dt.float32
NPDT = ml_dtypes.bfloat16

AF = mybir.ActivationFunctionType
OP = mybir.AluOpType


def build():
    nc = bacc.Bacc("TRN2", target_bir_lowering=False, debug=False,
                   num_devices=NCORES)

    qT_d = nc.dram_tensor("qT", [D, QH], DT, kind="ExternalInput").ap()
    kT_d = nc.dram_tensor("kT", [D, S], DT, kind="ExternalInput").ap()
    vT_d = nc.dram_tensor("vT", [D, S], DT, kind="ExternalInput").ap()
    wq_d = nc.dram_tensor("wq", [D, D], DT, kind="ExternalInput").ap()
    wk_d = nc.dram_tensor("wk", [D, D], DT, kind="ExternalInput").ap()
    wv_d = nc.dram_tensor("wv", [D, D], DT, kind="ExternalInput").ap()
    bq_d = nc.dram_tensor("bqc", [P, DJ], F32, kind="ExternalInput").ap()
    bk_d = nc.dram_tensor("bkc", [P, DJ], F32, kind="ExternalInput").ap()
    bv_d = nc.dram_tensor("bvb", [P, D], F32, kind="ExternalInput").ap()
    out_d = nc.dram_tensor("out", [QH, D], F32, kind="ExternalOutput").ap()

    def part3(ap):  # [(n p), d] -> [p, n, d]
        return ap.rearrange("(n p) d -> p n d", p=P)

    with tile.TileContext(nc) as tc:
        with (
            tc.tile_pool(name="persist", bufs=1) as pp,
            tc.tile_pool(name="xin", bufs=16) as xp,
            tc.tile_pool(name="win", bufs=16) as wp,
            tc.tile_pool(name="ev", bufs=2) as ep,
            tc.tile_pool(name="psum", bufs=2, space="PSUM") as psp,
        ):
            # constants
            bq_t = pp.tile([P, DJ], F32, tag="bq")
            nc.sync.dma_start(bq_t[:], bq_d[:])
            bk_t = pp.tile([P, DJ], F32, tag="bk")
            nc.sync.dma_start(bk_t[:], bk_d[:])
            bv_t = pp.tile([P, D], F32, tag="bv")
            nc.sync.dma_start(bv_t[:], bv_d[:])
            ones_t = pp.tile([P, 1], DT, tag="ones")
            nc.vector.memset(ones_t[:], 1.0)

            # persistent intermediates
            qT_proj = pp.tile([P, DJ, QH], DT, tag="qproj")   # (Q+bq)^T / 32
            kT_proj = pp.tile([P, DJ, S], DT, tag="kproj")    # (K+bk)^T
            expT = pp.tile([P, KJ, QH], DT, tag="expT")       # exp(scores)^T
            v_nat = pp.tile([P, KJ, D], DT, tag="vnat")       # value@wv (no bias)

            def load_w(src):
                # per-d-tile tiles: fine-grained deps let the first matmul
                # start after one 256KB chunk instead of the whole tensor
                out = []
                for di in range(DJ):
                    t = wp.tile([P, D], DT, tag="w")
                    nc.sync.dma_start(t[:], src[:, di, :])
                    out.append(t)
                return out

            def load_x(src, lo, width):
                out = []
                for di in range(DJ):
                    t = xp.tile([P, width], DT, tag="x")
                    nc.sync.dma_start(t[:], src[:, di, ds(lo, width)])
                    out.append(t)
                return out

            # ---- P1: Q projection -> qT_proj [d_out, q]
            wq_t = load_w(part3(wq_d))
            qT_in = load_x(part3(qT_d), 0, QH)
            for do in range(DJ):
                ps0 = psp.tile([P, NCH], F32, tag="psA")
                ps1 = psp.tile([P, NCH], F32, tag="psB")
                for di in range(DJ):
                    w_ap = wq_t[di][:, ts(do, P)]
                    nc.tensor.matmul(ps0[:], w_ap, qT_in[di][:, ds(0, NCH)],
                                     start=(di == 0), stop=(di == DJ - 1))
                    nc.tensor.matmul(ps1[:], w_ap, qT_in[di][:, ds(NCH, NCH)],
                                     start=(di == 0), stop=(di == DJ - 1))
                nc.vector.tensor_scalar(qT_proj[:, do, ds(0, NCH)], ps0[:],
                                        bq_t[:, ds(do, 1)], SCALE, OP.add, OP.mult)
                nc.vector.tensor_scalar(qT_proj[:, do, ds(NCH, NCH)], ps1[:],
                                        bq_t[:, ds(do, 1)], SCALE, OP.add, OP.mult)

            # ---- P2: K projection (full seq) -> kT_proj [d_out, s]
            # processed in sequence halves so the "x" input slots stay small
            # (QH-sized) and next-half loads overlap current-half compute
            wk_t = load_w(part3(wk_d))
            kT_r = part3(kT_d)
            for half in range(2):
                kT_in = load_x(kT_r, half * QH, QH)
                for do in range(DJ):
                    ps0 = psp.tile([P, NCH], F32, tag="psA")
                    ps1 = psp.tile([P, NCH], F32, tag="psB")
                    for di in range(DJ):
                        w_ap = wk_t[di][:, ts(do, P)]
                        nc.tensor.matmul(ps0[:], w_ap, kT_in[di][:, ds(0, NCH)],
                                         start=(di == 0), stop=(di == DJ - 1))
                        nc.tensor.matmul(ps1[:], w_ap, kT_in[di][:, ds(NCH, NCH)],
                                         start=(di == 0), stop=(di == DJ - 1))
                    nc.vector.tensor_scalar_add(
                        kT_proj[:, do, ds(half * 1024, NCH)], ps0[:],
                        bk_t[:, ds(do, 1)])
                    nc.vector.tensor_scalar_add(
                        kT_proj[:, do, ds(half * 1024 + NCH, NCH)], ps1[:],
                        bk_t[:, ds(do, 1)])

            # ---- P3: scores^T + exp -> expT [k, q]
            for kt in range(KJ):
                ps0 = psp.tile([P, NCH], F32, tag="psA")
                ps1 = psp.tile([P, NCH], F32, tag="psB")
                for di in range(DJ):
                    k_ap = kT_proj[:, di, ts(kt, P)]
                    nc.tensor.matmul(ps0[:], k_ap, qT_proj[:, di, ds(0, NCH)],
                                     start=(di == 0), stop=(di == DJ - 1))
                    nc.tensor.matmul(ps1[:], k_ap, qT_proj[:, di, ds(NCH, NCH)],
                                     start=(di == 0), stop=(di == DJ - 1))
                nc.scalar.activation(expT[:, kt, ds(0, NCH)], ps0[:], AF.Exp)
                nc.scalar.activation(expT[:, kt, ds(NCH, NCH)], ps1[:], AF.Exp)

            # ---- P4: V projection (full seq, natural layout, no bias)
            wv_t = load_w(part3(wv_d))
            vT_r = part3(vT_d)
            for half in range(2):
                vT_in = load_x(vT_r, half * QH, QH)
                for st in range(KJ // 2):
                    sg = half * (KJ // 2) + st
                    ps0 = psp.tile([P, NCH], F32, tag="psA")
                    ps1 = psp.tile([P, NCH], F32, tag="psB")
                    for di in range(DJ):
                        v_ap = vT_in[di][:, ts(st, P)]
                        nc.tensor.matmul(ps0[:], v_ap, wv_t[di][:, ds(0, NCH)],
                                         start=(di == 0), stop=(di == DJ - 1))
                        nc.tensor.matmul(ps1[:], v_ap, wv_t[di][:, ds(NCH, NCH)],
                                         start=(di == 0), stop=(di == DJ - 1))
                    nc.vector.tensor_copy(v_nat[:, sg, ds(0, NCH)], ps0[:])
                    nc.vector.tensor_copy(v_nat[:, sg, ds(NCH, NCH)], ps1[:])

            # ---- P5: AV + fused normalize/bias -> out
            for qt in range(QJ):
                po0 = psp.tile([P, NCH], F32, tag="psA")
                po1 = psp.tile([P, NCH], F32, tag="psB")
                psm = psp.tile([P, 1], F32, tag="psS")
                for kt in range(KJ):
                    e_ap = expT[:, kt, ts(qt, P)]
                    nc.tensor.matmul(po0[:], e_ap, v_nat[:, kt, ds(0, NCH)],
                                     start=(kt == 0), stop=(kt == KJ - 1))
                    nc.tensor.matmul(po1[:], e_ap, v_nat[:, kt, ds(NCH, NCH)],
                                     start=(kt == 0), stop=(kt == KJ - 1))
                    nc.tensor.matmul(psm[:], e_ap, ones_t[:],
                                     start=(kt == 0), stop=(kt == KJ - 1))
                recip = ep.tile([P, 1], F32, tag="recip")
                nc.vector.reciprocal(recip[:], psm[:])
                ot = ep.tile([P, D], F32, tag="out")
                nc.vector.scalar_tensor_tensor(
                    ot[:, ds(0, NCH)], po0[:], recip[:], bv_t[:, ds(0, NCH)],
                    OP.mult, OP.add)
                nc.vector.scalar_tensor_tensor(
                    ot[:, ds(NCH, NCH)], po1[:], recip[:], bv_t[:, ds(NCH, NCH)],
                    OP.mult, OP.add)
                nc.sync.dma_start(out_d[ts(qt, P), :], ot[:])

    nc.compile()
    return nc


_NC = None


def _get_nc():
    global _NC
    if _NC is None:
        _NC = build()
    return _NC


def _install_profile_hook():
    """The agent image's `antenv` lacks `axon_hooks`, so the boot-time NTFF
    profile hook install degrades silently. Recreate the registry module and
    install the ctypes-based hook so trace=True yields exec_time_ns."""
    import types
    try:
        from antenv.axon_hooks import get_axon_ntff_profile_hook  # noqa: F401
        return  # already present
    except ImportError:
        pass
    import antenv
    mod = types.ModuleType("antenv.axon_hooks")
    _hook = [None]
    mod.set_axon_ntff_profile_hook = lambda h: _hook.__setitem__(0, h)
    mod.get_axon_ntff_profile_hook = lambda: _hook[0]
    sys.modules["antenv.axon_hooks"] = mod
    antenv.axon_hooks = mod
    sys.path.insert(0, "/root/.axon_site")
    from trn_agent_boot.trn_boot import _ntff_profile_via_ctypes
    mod.set_axon_ntff_profile_hook(
        _ntff_profile_via_ctypes("/opt/axon/libaxon_pjrt.so"))


def _prep_in_maps(inputs):
    f32 = np.float32
    q = np.asarray(inputs["query"], f32)
    k = np.asarray(inputs["key"], f32)
    v = np.asarray(inputs["value"], f32)
    wq = np.ascontiguousarray(np.asarray(inputs["wq"], f32).astype(NPDT))
    wk = np.ascontiguousarray(np.asarray(inputs["wk"], f32).astype(NPDT))
    wv = np.ascontiguousarray(np.asarray(inputs["wv"], f32).astype(NPDT))
    bq = np.ascontiguousarray(np.asarray(inputs["bq"], f32).reshape(DJ, P).T)
    bk = np.ascontiguousarray(np.asarray(inputs["bk"], f32).reshape(DJ, P).T)
    bv = np.ascontiguousarray(
        np.broadcast_to(np.asarray(inputs["bv"], f32), (P, D)))

    in_maps = []
    for c in range(NCORES):
        b, h = divmod(c, 2)
        qT = np.ascontiguousarray(
            q[b, h * QH:(h + 1) * QH, :].astype(NPDT).T)
        kT = np.ascontiguousarray(k[b].astype(NPDT).T)
        vT = np.ascontiguousarray(v[b].astype(NPDT).T)
        in_maps.append({
            "qT": qT, "kT": kT, "vT": vT,
            "wq": wq, "wk": wk, "wv": wv,
            "bqc": bq, "bkc": bk, "bvb": bv,
        })
    return in_maps


def run(inputs, trace=False):
    """Returns (full_output [B,S,D] fp32, exec_time_ns or None)."""
    nc = _get_nc()
    in_maps = _prep_in_maps(inputs)
    if trace:
        _install_profile_hook()
    res = run_bass_kernel_spmd(nc, in_maps, list(range(NCORES)), trace=trace)
    out = np.empty((B, S, D), np.float32)
    for c in range(NCORES):
        b, h = divmod(c, 2)
        out[b, h * QH:(h + 1) * QH, :] = res.results[c]["out"]
    return out, res.exec_time_ns


def kernel(**inputs):
    return run(inputs, trace=False)[0]
